# revision 6
# baseline (speedup 1.0000x reference)
"""Single-head causal attention (B=4, T=4096, D=512, H=128) on 8 TRN2 NeuronCores.

Sharding: data-parallel over batch (4 batches x 2 cores). The two cores of a
batch split the 32 query tiles zig-zag style so causal work is balanced
(each core gets one long-context and one short-context tile per pair).
One SPMD program serves both core "types": per-slot k-tile counts are padded
to a shared compile-time schedule, and the causal boundary is applied with
per-core 0/1 mask tiles supplied as input data.

Per-core device program (all matmuls fp16 inputs -> fp32 PSUM):
  K^T = (wk^T @ x^T), V = x @ wv + bv, Q^T = (wq'^T @ xq^T), wq' = wq/sqrt(H)
  per slot group (4 slots, k-outer): S^T[k,q] = K^T_tile.T @ Q^T block
  P = exp(S^T) (no max subtraction: scores are O(5), fp16 holds exp fine),
  boundary tiles multiplied by 0/1 masks, O[q, 0:129] += P^T.T @ [V | 1]
  out = O[:, :128] * (1 / O[:, 128]).
The ones-column of V carries the softmax denominator through the same PSUM
accumulation, so no running max/sum bookkeeping is needed.
"""

import numpy as np
import ml_dtypes

B, T, D, H = 4, 4096, 512, 128
P = 128          # partitions / tile edge
DO = D // P      # contraction chunks (4)
NT = T // P      # k tiles per batch (32)
NS = 16          # query-tile slots per core
TQ = NS * P      # queries per core (2048)
KC = [32 - 2 * s for s in range(NS)]   # k-tiles processed per slot (desc)

_f16 = np.float16

_CACHE = {}


def _slot_qtile(core_type: int):
    """Global q-tile index handled by each slot, for core type 0/1."""
    out = []
    for s in range(NS):
        if s < 8:
            j = 31 - 2 * s - core_type          # long-context slots
        else:
            p = 15 - s
            j = 2 * p + core_type               # short-context slots
        out.append(j)
    return out


def _build_program():
    import concourse.tile as tile
    from concourse import bacc, mybir
    from concourse.bass import ts, ds

    f16 = mybir.dt.float16
    f32 = mybir.dt.float32
    Exp = mybir.ActivationFunctionType.Exp

    nc = bacc.Bacc("TRN2", target_bir_lowering=False, debug=False, num_devices=8)

    xT_d = nc.dram_tensor("xT", [P, DO, T], f16, kind="ExternalInput").ap()
    xqT_d = nc.dram_tensor("xqT", [P, DO, TQ], f16, kind="ExternalInput").ap()
    wq_d = nc.dram_tensor("wq", [P, DO, P], f16, kind="ExternalInput").ap()
    wk_d = nc.dram_tensor("wk", [P, DO, P], f16, kind="ExternalInput").ap()
    wv_d = nc.dram_tensor("wv", [P, DO, P], f16, kind="ExternalInput").ap()
    bq_d = nc.dram_tensor("bq", [P, 1], f32, kind="ExternalInput").ap()
    bk_d = nc.dram_tensor("bk", [P, 1], f32, kind="ExternalInput").ap()
    bvb_d = nc.dram_tensor("bvb", [P, P], f32, kind="ExternalInput").ap()
    msk_d = nc.dram_tensor("msk", [P, NS * 2 * P], f16, kind="ExternalInput").ap()
    out_d = nc.dram_tensor("out", [NS, P, P], f32, kind="ExternalOutput").ap()

    with tile.TileContext(nc) as tc:
        with tc.tile_pool(name="const", bufs=1) as cpool, \
             tc.tile_pool(name="data", bufs=1) as dpool:
            wq_sb = cpool.tile([P, DO, P], f16)
            wk_sb = cpool.tile([P, DO, P], f16)
            wv_sb = cpool.tile([P, DO, P], f16)
            bq_sb = cpool.tile([P, 1], f32)
            bk_sb = cpool.tile([P, 1], f32)
            bvb_sb = cpool.tile([P, P], f32)
            msk_sb = cpool.tile([P, NS * 2 * P], f16)
            for sb, d in [(wq_sb, wq_d), (wk_sb, wk_d), (wv_sb, wv_d),
                          (bq_sb, bq_d), (bk_sb, bk_d), (bvb_sb, bvb_d),
                          (msk_sb, msk_d)]:
                nc.sync.dma_start(sb[:], d)

            xT_sb = dpool.tile([P, DO, T], f16)
            xqT_sb = dpool.tile([P, DO, TQ], f16)
            KT_sb = dpool.tile([P, T], f16)
            QT_sb = dpool.tile([P, TQ], f16)
            V_sb = dpool.tile([P, NT, 130], f16)

            for st in range(T // 512):
                nc.sync.dma_start(xT_sb[:, :, ts(st, 512)], xT_d[:, :, ts(st, 512)])
            for st in range(TQ // 512):
                nc.sync.dma_start(xqT_sb[:, :, ts(st, 512)], xqT_d[:, :, ts(st, 512)])

            nc.vector.memset(V_sb[:, :, 128:130], 0.0)
            nc.vector.memset(V_sb[:, :, 128:129], 1.0)

            # ---- projections ----
            with tc.tile_pool(name="pproj", bufs=4, space="PSUM") as pp:
                for st in range(TQ // 512):     # Q^T strips
                    ps = pp.tile([P, 512], f32, tag="proj")
                    for o in range(DO):
                        nc.tensor.matmul(ps, wq_sb[:, o], xqT_sb[:, o, ts(st, 512)],
                                         start=(o == 0), stop=(o == DO - 1))
                    nc.vector.tensor_scalar_add(QT_sb[:, ts(st, 512)], ps, bq_sb)
                for st in range(T // 512):      # K^T strips
                    ps = pp.tile([P, 512], f32, tag="proj")
                    for o in range(DO):
                        nc.tensor.matmul(ps, wk_sb[:, o], xT_sb[:, o, ts(st, 512)],
                                         start=(o == 0), stop=(o == DO - 1))
                    nc.vector.tensor_scalar_add(KT_sb[:, ts(st, 512)], ps, bk_sb)
                    for tt in range(4 * st, 4 * st + 4):   # V tiles in this strip
                        ps_v = pp.tile([P, P], f32, tag="proj")
                        for o in range(DO):
                            nc.tensor.matmul(ps_v, xT_sb[:, o, ts(tt, P)], wv_sb[:, o],
                                             start=(o == 0), stop=(o == DO - 1))
                        nc.vector.tensor_add(V_sb[:, tt, 0:128], ps_v, bvb_sb)

            # ---- attention ----
            with tc.tile_pool(name="ps_s", bufs=3, space="PSUM") as ps_pool, \
                 tc.tile_pool(name="ps_o", bufs=5, space="PSUM") as po_pool, \
                 tc.tile_pool(name="sb_w", bufs=4) as wpool, \
                 tc.tile_pool(name="sb_f", bufs=3) as fpool:
                for g in range(4):
                    slots = list(range(4 * g, 4 * g + 4))
                    o_ps = {s: po_pool.tile([P, 129], f32, tag="oacc",
                                            name=f"o_acc_{s}") for s in slots}
                    c0 = KC[slots[0]]
                    for u in range(c0):
                        w = sum(1 for s in slots if KC[s] > u)
                        s_ps = ps_pool.tile([P, 512], f32, tag="sacc")
                        nc.tensor.matmul(s_ps[:, 0:w * P], KT_sb[:, ts(u, P)],
                                         QT_sb[:, ds(512 * g, w * P)],
                                         start=True, stop=True)
                        p_sb = wpool.tile([P, 512], f16, tag="ptile")
                        nc.scalar.activation(p_sb[:, 0:w * P], s_ps[:, 0:w * P], Exp)
                        for ci, s in enumerate(slots[:w]):
                            if u >= KC[s] - 2:
                                i = u - (KC[s] - 2)
                                nc.vector.tensor_mul(
                                    p_sb[:, ts(ci, P)], p_sb[:, ts(ci, P)],
                                    msk_sb[:, ds((2 * s + i) * P, P)])
                        for ci, s in enumerate(slots[:w]):
                            nc.tensor.matmul(o_ps[s], p_sb[:, ts(ci, P)],
                                             V_sb[:, u, 0:129],
                                             start=(u == 0), stop=(u == KC[s] - 1))
                            if u == KC[s] - 1:
                                rec = fpool.tile([P, 1], f32, tag="rec")
                                nc.vector.reciprocal(rec, o_ps[s][:, 128:129])
                                o_sb = fpool.tile([P, P], f32, tag="osb")
                                nc.vector.tensor_scalar_mul(o_sb, o_ps[s][:, 0:128], rec)
                                nc.sync.dma_start(out_d[s], o_sb)

    nc.compile()
    return nc


def _prep_core(core, x, wq, bq, wk, bk, wv, bv):
    b, ct = core // 2, core % 2
    qtiles = _slot_qtile(ct)
    scale = np.float32(1.0 / np.sqrt(H))

    def dchunk(a):  # [D, N] -> [P, DO, N] with d = o*P + p
        return np.ascontiguousarray(
            a.reshape(DO, P, -1).transpose(1, 0, 2)).astype(_f16)

    xT = x[b].T.astype(np.float32)                      # [D, T]
    qrows = np.concatenate([np.arange(j * P, (j + 1) * P) for j in qtiles])
    xqT = np.ascontiguousarray(xT[:, qrows])            # [D, TQ]

    msk = np.zeros((P, NS * 2 * P), dtype=np.float32)
    tri = np.triu(np.ones((P, P), dtype=np.float32))    # valid: qf >= kp
    for s in range(NS):
        j = qtiles[s]
        for i in range(2):
            u = KC[s] - 2 + i
            blk = msk[:, (2 * s + i) * P:(2 * s + i + 1) * P]
            if u < j:
                blk[:] = 1.0
            elif u == j:
                blk[:] = tri

    return {
        "xT": dchunk(xT),
        "xqT": dchunk(xqT),
        "wq": dchunk(wq * scale),
        "wk": dchunk(wk),
        "wv": dchunk(wv),
        "bq": (bq * scale).astype(np.float32).reshape(P, 1),
        "bk": bk.astype(np.float32).reshape(P, 1),
        "bvb": np.tile(bv.astype(np.float32), (P, 1)),
        "msk": msk.astype(_f16),
    }


def _fallback(x, mask, wq, bq, wk, bk, wv, bv):
    """Exact numpy path for inputs the specialized kernel doesn't cover."""
    out = np.empty((x.shape[0], x.shape[1], wq.shape[1]), dtype=np.float32)
    scale = np.float32(1.0 / np.sqrt(wq.shape[1]))
    for b in range(x.shape[0]):
        q = x[b] @ wq + bq
        k = x[b] @ wk + bk
        v = x[b] @ wv + bv
        s = (q @ k.T) * scale
        s = np.where(mask == 0, np.float32(-1e30), s)
        s -= s.max(axis=-1, keepdims=True)
        p = np.exp(s)
        p /= p.sum(axis=-1, keepdims=True)
        out[b] = p @ v
    return out


def kernel(**inputs):
    x = np.asarray(inputs["x"], dtype=np.float32)
    mask = np.asarray(inputs["mask"])
    wq = np.asarray(inputs["wq"], dtype=np.float32)
    bq = np.asarray(inputs["bq"], dtype=np.float32)
    wk = np.asarray(inputs["wk"], dtype=np.float32)
    bk = np.asarray(inputs["bk"], dtype=np.float32)
    wv = np.asarray(inputs["wv"], dtype=np.float32)
    bv = np.asarray(inputs["bv"], dtype=np.float32)

    causal = (x.shape == (B, T, D) and wq.shape == (D, H)
              and np.array_equal(mask, np.tril(np.ones((T, T), mask.dtype))))
    if not causal:
        return _fallback(x, mask, wq, bq, wk, bk, wv, bv)

    if "nc" not in _CACHE:
        _CACHE["nc"] = _build_program()
    nc = _CACHE["nc"]

    from concourse import bass_utils
    in_maps = [_prep_core(c, x, wq, bq, wk, bk, wv, bv) for c in range(8)]
    res = bass_utils.run_bass_kernel_spmd(nc, in_maps, core_ids=list(range(8)),
                                          **_CACHE.get("run_kwargs", {}))
    _CACHE["last_result"] = res

    out = np.empty((B, T, H), dtype=np.float32)
    for c in range(8):
        b, ct = c // 2, c % 2
        qtiles = _slot_qtile(ct)
        oc = res.results[c]["out"]          # [NS, P, P]
        for s, j in enumerate(qtiles):
            out[b, j * P:(j + 1) * P, :] = oc[s]
    return out


# revision 7
# speedup vs baseline: 1.1429x; 1.1429x over previous
"""Single-head causal attention (B=4, T=4096, D=512, H=128) on 8 TRN2 NeuronCores.

Sharding: data-parallel over batch (4 batches x 2 cores). The two cores of a
batch split the 32 query tiles zig-zag style so causal work is balanced
(each core gets one long-context and one short-context tile per pair).
One SPMD program serves both core "types": per-slot k-tile counts are padded
to a shared compile-time schedule, and the causal boundary is applied with
per-core 0/1 mask tiles supplied as input data.

Per-core device program (all matmuls fp16 inputs -> fp32 PSUM):
  K^T = (wk^T @ x^T), V = x @ wv + bv, Q^T = (wq'^T @ xq^T), wq' = wq/sqrt(H)
  per slot group (4 slots, k-outer): S^T[k,q] = K^T_tile.T @ Q^T block
  P = exp(S^T) (no max subtraction: scores are O(5), fp16 holds exp fine),
  boundary tiles multiplied by 0/1 masks, O[q, 0:129] += P^T.T @ [V | 1]
  out = O[:, :128] * (1 / O[:, 128]).
The ones-column of V carries the softmax denominator through the same PSUM
accumulation, so no running max/sum bookkeeping is needed.
"""

import numpy as np
import ml_dtypes

B, T, D, H = 4, 4096, 512, 128
P = 128          # partitions / tile edge
DO = D // P      # contraction chunks (4)
NT = T // P      # k tiles per batch (32)
NS = 16          # query-tile slots per core
TQ = NS * P      # queries per core (2048)
KC = [32 - 2 * s for s in range(NS)]   # k-tiles processed per slot (desc)

_f16 = np.float16

_CACHE = {}


def _slot_qtile(core_type: int):
    """Global q-tile index handled by each slot, for core type 0/1."""
    out = []
    for s in range(NS):
        if s < 8:
            j = 31 - 2 * s - core_type          # long-context slots
        else:
            p = 15 - s
            j = 2 * p + core_type               # short-context slots
        out.append(j)
    return out


def _build_program():
    import concourse.tile as tile
    from concourse import bacc, mybir
    from concourse.bass import ts, ds

    f16 = mybir.dt.float16
    f32 = mybir.dt.float32
    Exp = mybir.ActivationFunctionType.Exp

    nc = bacc.Bacc("TRN2", target_bir_lowering=False, debug=False, num_devices=8)

    xT_d = nc.dram_tensor("xT", [P, DO, T], f16, kind="ExternalInput").ap()
    xqT_d = nc.dram_tensor("xqT", [P, DO, TQ], f16, kind="ExternalInput").ap()
    wq_d = nc.dram_tensor("wq", [P, DO, P], f16, kind="ExternalInput").ap()
    wk_d = nc.dram_tensor("wk", [P, DO, P], f16, kind="ExternalInput").ap()
    wv_d = nc.dram_tensor("wv", [P, DO, P], f16, kind="ExternalInput").ap()
    bq_d = nc.dram_tensor("bq", [P, 1], f32, kind="ExternalInput").ap()
    bk_d = nc.dram_tensor("bk", [P, 1], f32, kind="ExternalInput").ap()
    bvb_d = nc.dram_tensor("bvb", [P, P], f32, kind="ExternalInput").ap()
    msk_d = nc.dram_tensor("msk", [P, NS * 2 * P], f16, kind="ExternalInput").ap()
    out_d = nc.dram_tensor("out", [NS, P, P], f32, kind="ExternalOutput").ap()

    NSTRIP = T // 512          # 8 key strips
    NQSTRIP = TQ // 512        # 4 query strips (one per slot group)

    with tile.TileContext(nc) as tc:
        with tc.tile_pool(name="const", bufs=1) as cpool, \
             tc.tile_pool(name="data", bufs=1) as dpool:
            wq_sb = cpool.tile([P, DO, P], f16)
            wk_sb = cpool.tile([P, DO, P], f16)
            wv_sb = cpool.tile([P, DO, P], f16)
            bq_sb = cpool.tile([P, 1], f32)
            bk_sb = cpool.tile([P, 1], f32)
            bvb_sb = cpool.tile([P, P], f32)
            msk_sb = cpool.tile([P, NS * 2 * P], f16)
            for sb, d in [(wq_sb, wq_d), (wk_sb, wk_d), (wv_sb, wv_d),
                          (bq_sb, bq_d), (bk_sb, bk_d), (bvb_sb, bvb_d)]:
                nc.sync.dma_start(sb[:], d)

            # per-strip tiles so dependencies stay fine-grained: attention on
            # early k-tiles runs while later x strips are still in flight.
            xq_t = [dpool.tile([P, DO, 512], f16, name=f"xq_{i}")
                    for i in range(NQSTRIP)]
            xt_t = [dpool.tile([P, DO, 512], f16, name=f"xt_{i}")
                    for i in range(NSTRIP)]
            qt_t = [dpool.tile([P, 512], f16, name=f"qt_{i}")
                    for i in range(NQSTRIP)]
            kt_t = [dpool.tile([P, 512], f16, name=f"kt_{i}")
                    for i in range(NSTRIP)]
            v_t = [dpool.tile([P, 130], f16, name=f"v_{i}") for i in range(NT)]

            for st in range(NQSTRIP):
                nc.sync.dma_start(xq_t[st][:], xqT_d[:, :, ts(st, 512)])
            for st in range(NSTRIP):
                nc.sync.dma_start(xt_t[st][:], xT_d[:, :, ts(st, 512)])
            nc.sync.dma_start(msk_sb[:], msk_d)
            for tt in range(NT):
                nc.vector.memset(v_t[tt][:, 128:129], 1.0)

            # ---- projections ----
            with tc.tile_pool(name="pproj", bufs=4, space="PSUM") as pp:
                for st in range(NQSTRIP):     # Q^T strips
                    ps = pp.tile([P, 512], f32, tag="proj", name=f"psq_{st}")
                    for o in range(DO):
                        nc.tensor.matmul(ps, wq_sb[:, o], xq_t[st][:, o],
                                         start=(o == 0), stop=(o == DO - 1))
                    nc.vector.tensor_scalar_add(qt_t[st][:], ps, bq_sb)
                for st in range(NSTRIP):      # K^T strips + V tiles per strip
                    ps = pp.tile([P, 512], f32, tag="proj", name=f"psk_{st}")
                    for o in range(DO):
                        nc.tensor.matmul(ps, wk_sb[:, o], xt_t[st][:, o],
                                         start=(o == 0), stop=(o == DO - 1))
                    nc.vector.tensor_scalar_add(kt_t[st][:], ps, bk_sb)
                    for j in range(4):
                        tt = 4 * st + j
                        ps_v = pp.tile([P, P], f32, tag="proj", name=f"psv_{tt}")
                        for o in range(DO):
                            nc.tensor.matmul(ps_v, xt_t[st][:, o, ts(j, P)],
                                             wv_sb[:, o],
                                             start=(o == 0), stop=(o == DO - 1))
                        nc.vector.tensor_add(v_t[tt][:, 0:128], ps_v, bvb_sb)

            # ---- attention ----
            with tc.tile_pool(name="ps_s", bufs=2, space="PSUM") as ps_pool, \
                 tc.tile_pool(name="ps_o", bufs=4, space="PSUM") as po_pool, \
                 tc.tile_pool(name="sb_w", bufs=4) as wpool, \
                 tc.tile_pool(name="sb_f", bufs=3) as fpool:
                for g in range(4):
                    slots = list(range(4 * g, 4 * g + 4))
                    o_ps = {s: po_pool.tile([P, 129], f32, tag="oacc",
                                            name=f"o_acc_{s}") for s in slots}
                    c0 = KC[slots[0]]
                    for up in range(c0 // 2):      # k-tile pairs
                        u0 = 2 * up
                        w = sum(1 for s in slots if KC[s] > u0)
                        s_ps = ps_pool.tile([P, 2, 512], f32, tag="sacc",
                                            name=f"s_{g}_{up}")
                        for j in range(2):
                            nc.tensor.matmul(s_ps[:, j, 0:w * P],
                                             kt_t[(u0 + j) // 4][:, ts((u0 + j) % 4, P)],
                                             qt_t[g][:, 0:w * P],
                                             start=True, stop=True)
                        p_sb = wpool.tile([P, 2, 512], f16, tag="ptile",
                                          name=f"p_{g}_{up}")
                        nc.scalar.activation(p_sb[:, :, 0:w * P],
                                             s_ps[:, :, 0:w * P], Exp)
                        for ci, s in enumerate(slots[:w]):
                            if u0 == KC[s] - 2:    # this pair is s's boundary
                                nc.vector.tensor_mul(
                                    p_sb[:, :, ts(ci, P)], p_sb[:, :, ts(ci, P)],
                                    msk_sb[:, 2 * s * P:(2 * s + 2) * P]
                                    .rearrange("p (i q) -> p i q", i=2))
                        for j in range(2):
                            u = u0 + j
                            for ci, s in enumerate(slots[:w]):
                                nc.tensor.matmul(o_ps[s], p_sb[:, j, ts(ci, P)],
                                                 v_t[u][:, 0:129],
                                                 start=(u == 0),
                                                 stop=(u == KC[s] - 1))
                                if u == KC[s] - 1:
                                    rec = fpool.tile([P, 1], f32, tag="rec",
                                                     name=f"rec_{s}")
                                    nc.vector.reciprocal(rec, o_ps[s][:, 128:129])
                                    o_sb = fpool.tile([P, P], f32, tag="osb",
                                                      name=f"osb_{s}")
                                    nc.vector.tensor_scalar_mul(
                                        o_sb, o_ps[s][:, 0:128], rec)
                                    nc.sync.dma_start(out_d[s], o_sb)

    nc.compile()
    return nc


def _prep_core(core, x, wq, bq, wk, bk, wv, bv):
    b, ct = core // 2, core % 2
    qtiles = _slot_qtile(ct)
    scale = np.float32(1.0 / np.sqrt(H))

    def dchunk(a):  # [D, N] -> [P, DO, N] with d = o*P + p
        return np.ascontiguousarray(
            a.reshape(DO, P, -1).transpose(1, 0, 2)).astype(_f16)

    xT = x[b].T.astype(np.float32)                      # [D, T]
    qrows = np.concatenate([np.arange(j * P, (j + 1) * P) for j in qtiles])
    xqT = np.ascontiguousarray(xT[:, qrows])            # [D, TQ]

    msk = np.zeros((P, NS * 2 * P), dtype=np.float32)
    tri = np.triu(np.ones((P, P), dtype=np.float32))    # valid: qf >= kp
    for s in range(NS):
        j = qtiles[s]
        for i in range(2):
            u = KC[s] - 2 + i
            blk = msk[:, (2 * s + i) * P:(2 * s + i + 1) * P]
            if u < j:
                blk[:] = 1.0
            elif u == j:
                blk[:] = tri

    return {
        "xT": dchunk(xT),
        "xqT": dchunk(xqT),
        "wq": dchunk(wq * scale),
        "wk": dchunk(wk),
        "wv": dchunk(wv),
        "bq": (bq * scale).astype(np.float32).reshape(P, 1),
        "bk": bk.astype(np.float32).reshape(P, 1),
        "bvb": np.tile(bv.astype(np.float32), (P, 1)),
        "msk": msk.astype(_f16),
    }


def _fallback(x, mask, wq, bq, wk, bk, wv, bv):
    """Exact numpy path for inputs the specialized kernel doesn't cover."""
    out = np.empty((x.shape[0], x.shape[1], wq.shape[1]), dtype=np.float32)
    scale = np.float32(1.0 / np.sqrt(wq.shape[1]))
    for b in range(x.shape[0]):
        q = x[b] @ wq + bq
        k = x[b] @ wk + bk
        v = x[b] @ wv + bv
        s = (q @ k.T) * scale
        s = np.where(mask == 0, np.float32(-1e30), s)
        s -= s.max(axis=-1, keepdims=True)
        p = np.exp(s)
        p /= p.sum(axis=-1, keepdims=True)
        out[b] = p @ v
    return out


def kernel(**inputs):
    x = np.asarray(inputs["x"], dtype=np.float32)
    mask = np.asarray(inputs["mask"])
    wq = np.asarray(inputs["wq"], dtype=np.float32)
    bq = np.asarray(inputs["bq"], dtype=np.float32)
    wk = np.asarray(inputs["wk"], dtype=np.float32)
    bk = np.asarray(inputs["bk"], dtype=np.float32)
    wv = np.asarray(inputs["wv"], dtype=np.float32)
    bv = np.asarray(inputs["bv"], dtype=np.float32)

    causal = (x.shape == (B, T, D) and wq.shape == (D, H)
              and np.array_equal(mask, np.tril(np.ones((T, T), mask.dtype))))
    if not causal:
        return _fallback(x, mask, wq, bq, wk, bk, wv, bv)

    if "nc" not in _CACHE:
        _CACHE["nc"] = _build_program()
    nc = _CACHE["nc"]

    from concourse import bass_utils
    in_maps = [_prep_core(c, x, wq, bq, wk, bk, wv, bv) for c in range(8)]
    res = bass_utils.run_bass_kernel_spmd(nc, in_maps, core_ids=list(range(8)),
                                          **_CACHE.get("run_kwargs", {}))
    _CACHE["last_result"] = res

    out = np.empty((B, T, H), dtype=np.float32)
    for c in range(8):
        b, ct = c // 2, c % 2
        qtiles = _slot_qtile(ct)
        oc = res.results[c]["out"]          # [NS, P, P]
        for s, j in enumerate(qtiles):
            out[b, j * P:(j + 1) * P, :] = oc[s]
    return out
